# revision 8
# baseline (speedup 1.0000x reference)
"""Trainium2 Bass kernel for nn_AffineExponential.

Computes, for each sample b:
    y_b   = expm(t_b * W) @ x_b + t_b * bias
    ljd_b = t_b * diag(W)

Key identity: expm(t W) x = sum_k (t^k / k!) W^k x, so instead of per-sample
matrix exponentials we run one shared chain of [128, B] matmuls with a scaled
recurrence  U_0 = X^T,  U_{k+1} = (W @ U_k) * t / (k+1)  and  y^T = sum_k U_k.
The per-column (per-sample) t scaling fuses into a single scalar_tensor_tensor
op per chain step. K=12 terms reaches the fp32 floor (spectral radius of W is
~1.08, t in [0,1); term k <= 1.08^k/k!).

Sharding: pure data-parallel over the batch dim, 8 cores x 512 samples.
weight/bias replicated. All dims hardcoded per the harness contract.
"""

import sys
from contextlib import ExitStack

import numpy as np

for _p in ("/opt/trn_rl_repo", "/root/.axon_site/_ro/trn_rl_repo"):
    if _p not in sys.path:
        sys.path.append(_p)

def _ensure_ntff_hook_module():
    """The agent image's antenv lacks axon_hooks; provide it so
    run_bass_kernel_spmd's trace=True path can profile. No-op if present."""
    import types
    try:
        import antenv.axon_hooks  # noqa: F401
        return
    except ImportError:
        pass
    mod = types.ModuleType("antenv.axon_hooks")
    _state = {"hook": None}
    mod.set_axon_ntff_profile_hook = lambda h: _state.__setitem__("hook", h)
    mod.get_axon_ntff_profile_hook = lambda: _state["hook"]
    sys.modules["antenv.axon_hooks"] = mod
    try:
        from trn_agent_boot.trn_boot import _ntff_profile_via_ctypes
        mod.set_axon_ntff_profile_hook(
            _ntff_profile_via_ctypes("/opt/axon/libaxon_pjrt.so"))
    except Exception:
        pass


_ensure_ntff_hook_module()

import concourse.bass as bass
import concourse.tile as tile
from concourse import mybir
from concourse.bass_utils import run_bass_kernel_spmd
from concourse.masks import make_identity

B, D = 4096, 128
N_CORES = 8
B_LOC = B // N_CORES  # 512
K_TERMS = 12  # terms 0..11; term k magnitude <= (t*rho)^k/k!, rho ~ 1.08
F32 = mybir.dt.float32
MULT = mybir.AluOpType.mult
ADD = mybir.AluOpType.add


def _hoist_waits(nc: bass.Bass) -> int:
    """Move semaphore waits off instructions onto standalone EventSemaphore
    instructions. This walrus build rejects any wait attached to a Matmult
    (S3_LW struct) and allows at most one elsewhere ("Too many sync wait
    commands"); a preceding same-engine wait instruction is equivalent."""
    n = 0
    for f in nc.m.functions:
        for blk in f.blocks:
            il = blk.instructions
            i = 0
            while i < len(il):
                ins = il[i]
                si = ins.sync_info
                if si is None or not si.on_wait:
                    i += 1
                    continue
                keep = 0 if ins.__class__.__name__ in ("InstMatmult", "InstMatmultMx") else 1
                waits = list(si.on_wait)
                if len(waits) <= keep:
                    i += 1
                    continue
                hoisted = waits[: len(waits) - keep]
                si.on_wait = waits[len(waits) - keep:]
                for w in hoisted:
                    wi = mybir.InstEventSemaphore(
                        name=f"W-hoist-{n}", engine=ins.engine, ins=[], outs=[])
                    wi.sync_info = type(si)(on_wait=[w], on_update=[])
                    il.insert(i, wi)
                    n += 1
                    i += 1
                i += 1
    return n


def _build_program() -> bass.Bass:
    nc = bass.Bass("TRN2", target_bir_lowering=False, debug=False,
                   enable_asserts=False, num_devices=N_CORES)

    x_d = nc.dram_tensor("x", [B_LOC, D], F32, kind="ExternalInput").ap()
    t_d = nc.dram_tensor("t", [B_LOC, 1], F32, kind="ExternalInput").ap()
    w_d = nc.dram_tensor("w", [D, D], F32, kind="ExternalInput").ap()
    b_d = nc.dram_tensor("b", [1, D], F32, kind="ExternalInput").ap()
    y_d = nc.dram_tensor("y", [B_LOC, D], F32, kind="ExternalOutput").ap()
    ljd_d = nc.dram_tensor("ljd", [B_LOC, D], F32, kind="ExternalOutput").ap()

    NT = B_LOC // D  # 4 batch tiles of 128
    HALF = B_LOC // 2  # 256: chain runs as two independent column-halves

    with tile.TileContext(nc) as tc, ExitStack() as ctx:
        const = ctx.enter_context(tc.tile_pool(name="const", bufs=1))
        upool = ctx.enter_context(tc.tile_pool(name="u", bufs=6))
        obuf = ctx.enter_context(tc.tile_pool(name="obuf", bufs=4))
        ps_sm = ctx.enter_context(tc.tile_pool(name="ps_sm", bufs=2, space="PSUM"))
        ps_wide = ctx.enter_context(tc.tile_pool(name="ps_wide", bufs=1, space="PSUM"))
        ps_chain = ctx.enter_context(tc.tile_pool(name="ps_chain", bufs=3, space="PSUM"))
        ps_out = ctx.enter_context(tc.tile_pool(name="ps_out", bufs=2, space="PSUM"))

        # ---- constants / loads ----
        ident = const.tile([D, D], F32, tag="ident")
        make_identity(nc, ident)
        ones = const.tile([D, D], F32, tag="ones")
        nc.gpsimd.memset(ones, 1.0)

        x_bm = const.tile([D, NT, D], F32, tag="x_bm")
        nc.sync.dma_start(x_bm, x_d.rearrange("(m p) i -> p m i", p=D))
        w_sb = const.tile([D, D], F32, tag="w_sb")
        nc.sync.dma_start(w_sb, w_d)
        t_row = const.tile([1, B_LOC], F32, tag="t_row")
        nc.sync.dma_start(t_row, t_d.rearrange("a b -> b a"))
        bias_row = const.tile([1, D], F32, tag="bias_row")
        nc.sync.dma_start(bias_row, b_d)

        # ---- layout transposes: XT = x^T (feature-major), WT = W^T ----
        xt = const.tile([D, B_LOC], F32, tag="xt")
        for m in range(NT):
            ps = ps_sm.tile([D, D], F32, tag="ps_sm")
            nc.tensor.transpose(ps, x_bm[:, m, :], ident)
            nc.scalar.copy(xt[:, bass.ts(m, D)], ps)
        wt = const.tile([D, D], F32, tag="wt")
        ps = ps_sm.tile([D, D], F32, tag="ps_sm")
        nc.tensor.transpose(ps, w_sb, ident)
        nc.scalar.copy(wt, ps)

        # ---- T_rep[i, b] = t_b (broadcast across partitions via rank-1 matmul)
        t_rep = const.tile([D, B_LOC], F32, tag="t_rep")
        psT = ps_wide.tile([D, B_LOC], F32, tag="ps_wide")
        nc.tensor.matmul(psT, ones[0:1, :], t_row)
        nc.scalar.copy(t_rep, psT)

        # ---- diag(W) as a row: ones_col^T @ (W .* I) ----
        # wi on gpsimd so this matmul's inputs live on one semaphore domain
        wi = const.tile([D, D], F32, tag="wi")
        nc.gpsimd.tensor_mul(wi, w_sb, ident)
        ps = ps_sm.tile([D, D], F32, tag="ps_sm")
        nc.tensor.matmul(ps[0:1, :], ones[:, 0:1], wi)
        diag_row = const.tile([1, D], F32, tag="diag_row")
        nc.scalar.copy(diag_row, ps[0:1, :])

        # ---- Y init: Y = U_0 (term 0). bias x t folds into the final
        # transpose matmuls as a PSUM-accumulated rank-1 update, so y_fm has a
        # single writer engine (gpsimd) throughout.
        y_fm = const.tile([D, B_LOC], F32, tag="y_fm")
        nc.gpsimd.tensor_copy(y_fm, xt)

        # ---- Taylor chain: U_{k+1} = (W @ U_k) * t / (k+1); Y += U_{k+1} ----
        cur = [xt[:, 0:HALF], xt[:, HALF:B_LOC]]
        for k in range(1, K_TERMS):
            for h in range(2):
                sl = slice(h * HALF, (h + 1) * HALF)
                psc = ps_chain.tile([D, HALF], F32, tag="ps_chain")
                nc.tensor.matmul(psc, wt, cur[h])
                u_next = upool.tile([D, HALF], F32, tag="u")
                nc.vector.scalar_tensor_tensor(out=u_next, in0=psc,
                                               scalar=float(1.0 / k),
                                               in1=t_rep[:, sl],
                                               op0=MULT, op1=MULT)
                nc.gpsimd.tensor_add(y_fm[:, sl], y_fm[:, sl], u_next)
                cur[h] = u_next[:]

        # ---- outputs: transpose Y back to batch-major (with bias x t
        # accumulated into the same PSUM bank); ljd = t x diag(W) ----
        for m in range(NT):
            ps = ps_out.tile([D, D], F32, tag="ps_out")
            nc.tensor.transpose(ps, y_fm[:, bass.ts(m, D)], ident)
            nc.tensor.matmul(ps, t_row[0:1, bass.ts(m, D)], bias_row,
                             start=False, stop=True, skip_group_check=True)
            yo = obuf.tile([D, D], F32, tag="yo")
            nc.scalar.copy(yo, ps)
            nc.sync.dma_start(y_d[bass.ts(m, D), :], yo)

            psl = ps_out.tile([D, D], F32, tag="ps_out")
            nc.tensor.matmul(psl, t_row[0:1, bass.ts(m, D)], diag_row)
            lo = obuf.tile([D, D], F32, tag="lo")
            nc.scalar.copy(lo, psl)
            nc.sync.dma_start(ljd_d[bass.ts(m, D), :], lo)

    _hoist_waits(nc)
    return nc


_CACHE: dict = {}


def _run(x, t, weight, bias, trace=False, **trace_kw):
    if "nc" not in _CACHE:
        _CACHE["nc"] = _build_program()
    nc = _CACHE["nc"]
    x = np.ascontiguousarray(x, dtype=np.float32)
    t = np.ascontiguousarray(t, dtype=np.float32)
    w = np.ascontiguousarray(weight, dtype=np.float32)
    b = np.ascontiguousarray(bias, dtype=np.float32).reshape(1, D)
    in_maps = [
        {"x": x[i * B_LOC:(i + 1) * B_LOC], "t": t[i * B_LOC:(i + 1) * B_LOC],
         "w": w, "b": b}
        for i in range(N_CORES)
    ]
    res = run_bass_kernel_spmd(nc, in_maps, list(range(N_CORES)),
                               trace=trace, **trace_kw)
    y = np.concatenate([res.results[i]["y"] for i in range(N_CORES)], axis=0)
    ljd = np.concatenate([res.results[i]["ljd"] for i in range(N_CORES)], axis=0)
    return (y, ljd), res


def kernel(x, t, weight, bias):
    (y, ljd), _ = _run(x, t, weight, bias, trace=False)
    return y, ljd
